# revision 1
# baseline (speedup 1.0000x reference)
"""Trainium2 Bass kernel: 2-layer GCN (GCNConv -> ReLU -> GCNConv).

Math:  S = D^-1/2 (A + I) D^-1/2  (A from edge_index, self-loops appended)
       out = S @ relu(S @ x @ W1 + b1) @ W2 + b2
Using linearity, aggregate-then-matmul per layer with u = Dis*x:
       agg1 = A' @ u + u            (A' = adjacency without self-loops)
       h~   = Dis^2 * relu(agg1 @ W1 (+ b1))   (= Dis * h1, stored fp16)
       agg2 = A' @ h~ + h~
       out  = Dis * (agg2 @ W2) (+ b2)
The Dis row-scaling commutes with the dense matmul, so it is folded into a
single ScalarE activation per tile (scale = dis^2 resp. dis, func=relu/copy).

Distribution: nodes sharded over 8 NeuronCores.  Per layer, each core
scatter-adds incoming-edge source rows per 128-target tile with TensorE
matmuls against one-hot matrices (built on VectorE via is_equal vs iota, one
tile ahead of use), transposes via TensorE and applies the dense weight
matmul in fp16, all on a 2-deep software pipeline (PE order: aggregate(t),
transpose(t-1), dense(t-2)) so the TensorE stream never stalls and stays at
the warm HAM clock.
  Layer 1: gather indices are static and the source data (Dis*x, fp16) is a
  kernel input, so the HOST pre-expands the gathered stream into edge order;
  the device streams it sequentially over HWDGE at full HBM bandwidth.
  Layer 2: fp16 activations are AllGathered in THREE pieces (31/17/1 tiles —
  the last one tiny so almost no AllGather latency is exposed at the layer
  boundary), then fetched per-edge with gpsimd dma_gather.  Descriptor
  generation is spread over all 4 SWDGE queues (round-robin per call) so all
  8 Q7 cores generate descriptors in parallel instead of serializing on
  cores 0/1; gather indices are pre-sorted for HBM locality.
"""

import os
import numpy as np

NC_CORES = 8
TILE_P = 128
N_QUEUES = 4


def _round_up(v, m):
    return (v + m - 1) // m * m


def _prep_host(x, edge_index):
    """Partition + pad the graph; build per-core stream/gather metadata."""
    x = np.asarray(x, dtype=np.float32)
    edge_index = np.asarray(edge_index)
    N, F = x.shape
    assert N % NC_CORES == 0, (N, NC_CORES)
    npc = N // NC_CORES
    npc_pad = _round_up(npc, TILE_P)
    ntiles = npc_pad // TILE_P
    n_pad = NC_CORES * npc_pad

    loops = np.arange(N, dtype=np.int64)
    # edges WITHOUT self-loops (self term handled on-device)
    row = edge_index[0].astype(np.int64)
    col = edge_index[1].astype(np.int64)
    # degree WITH self-loops (as the reference computes it)
    deg = np.bincount(np.concatenate([col, loops]), minlength=N).astype(np.float64)
    dis = np.where(deg > 0, 1.0 / np.sqrt(deg), 0.0).astype(np.float32)

    src_pad = (row // npc) * npc_pad + (row % npc)
    tgt_core = (col // npc).astype(np.int64)
    tgt_loc = col % npc
    tile_of = tgt_loc // TILE_P
    toff_of = (tgt_loc % TILE_P).astype(np.float32)

    # ---- layer 1: host-expanded stream, grouped by (core, tile) ----
    key1 = tgt_core * ntiles + tile_of
    cnt1 = np.bincount(key1, minlength=NC_CORES * ntiles).reshape(
        NC_CORES, ntiles)
    C1 = _round_up(cnt1.max(axis=0), TILE_P)  # padded counts [ntiles]
    tot1 = int(C1.sum())
    totch1 = tot1 // TILE_P
    nch1 = (C1 // TILE_P).astype(np.int64)
    cs1 = np.zeros(ntiles, dtype=np.int64)
    np.cumsum(nch1[:-1], out=cs1[1:])

    o1 = np.argsort(key1, kind="stable")
    src1_s, toff1_s = src_pad[o1], toff_of[o1]
    g1start = np.zeros(NC_CORES * ntiles + 1, dtype=np.int64)
    np.cumsum(cnt1.reshape(-1), out=g1start[1:])

    # ---- layer 2: dma_gather, grouped by (core, tile, piece) ----
    # pieces are TILE ranges of each core's shard: a large one that
    # AllGathers mid-layer-1, a medium one near the end, and a 1-tile one so
    # the final AllGather latency at the layer boundary is tiny.  Every
    # piece must be int16-indexable: 8 * piece_rows <= 32767.
    ta = min(ntiles, 32767 // (NC_CORES * TILE_P))
    tbr = ntiles - ta
    if tbr >= 1:
        bounds_t = [0, ta, ntiles]
    else:
        bounds_t = [0, ntiles]
    NP = len(bounds_t) - 1
    bounds_r = [b * TILE_P for b in bounds_t]
    rows_p = [bounds_r[i + 1] - bounds_r[i] for i in range(NP)]
    for r in rows_p:
        assert NC_CORES * r <= 32767, (rows_p,)

    src_core = row // npc
    src_loc = row % npc
    piece = np.searchsorted(np.asarray(bounds_r[1:NP]), src_loc, side="right")
    key2 = key1 * NP + piece
    cnt2 = np.bincount(key2, minlength=NC_CORES * ntiles * NP).reshape(
        NC_CORES, ntiles, NP)
    C2 = cnt2.max(axis=0)
    C2 = np.where(C2 > 0, _round_up(C2, TILE_P), 0)  # [ntiles, NP]
    tot2 = int(C2.sum())
    cP = (C2 // TILE_P).astype(np.int64)  # chunks per (tile, piece)
    nch2 = cP.sum(axis=1)
    cs2 = np.zeros(ntiles, dtype=np.int64)
    np.cumsum(nch2[:-1], out=cs2[1:])

    start_r = np.asarray([bounds_r[p] for p in range(NP)])
    rows_arr = np.asarray(rows_p)
    piece_idx = src_core * rows_arr[piece] + (src_loc - start_r[piece])
    # secondary sort by source index: the one-hot P absorbs any within-group
    # permutation, and ascending gather addresses improve HBM locality
    o2 = np.lexsort((piece_idx, key2))
    src2_s, toff2_s = piece_idx[o2], toff_of[o2]
    g2start = np.zeros(NC_CORES * ntiles * NP + 1, dtype=np.int64)
    np.cumsum(cnt2.reshape(-1), out=g2start[1:])

    # padded fp16 Dis*x; per-core local shard wrapped [128, ntiles, F]
    xs = (dis[:, None] * x).astype(np.float16)
    xs_pad = np.zeros((NC_CORES, npc_pad, F), dtype=np.float16)
    xs_pad[:, :npc] = xs.reshape(NC_CORES, npc, F)
    xsl = np.ascontiguousarray(
        xs_pad.reshape(NC_CORES, ntiles, TILE_P, F).transpose(0, 2, 1, 3))
    xs_flat = xs_pad.reshape(n_pad, F)

    g1 = np.zeros((NC_CORES, 128, totch1, F), dtype=np.float16)
    toff1 = np.full((NC_CORES, tot1), -1.0, dtype=np.float32)
    idx2 = np.zeros((NC_CORES, max(tot2, 16)), dtype=np.int16)
    toff2 = np.full((NC_CORES, max(tot2, TILE_P)), -1.0, dtype=np.float32)
    for p in range(NC_CORES):
        off = 0
        for t in range(ntiles):
            g = p * ntiles + t
            a, b = g1start[g], g1start[g + 1]
            n = b - a
            blk = g1[p, :, cs1[t]:cs1[t] + nch1[t], :]
            j = np.arange(n)
            # stream row j -> partition j%128, chunk j//128
            blk[j % 128, j // 128] = xs_flat[src1_s[a:b]]
            toff1[p, off:off + n] = toff1_s[a:b]
            off += C1[t]
        off = 0
        for t in range(ntiles):
            for h in range(NP):
                c = int(C2[t, h])
                if c == 0:
                    continue
                g = (p * ntiles + t) * NP + h
                a, b = g2start[g], g2start[g + 1]
                n = b - a
                s = src2_s[a:b]
                assert n <= c and (s >= 0).all() and (s < 32767).all()
                idx2[p, off:off + n] = s.astype(np.int16)
                toff2[p, off:off + n] = toff2_s[a:b]
                off += c

    tot2c = max(tot2, 16)
    idx2_w = np.ascontiguousarray(
        np.tile(idx2.reshape(NC_CORES, tot2c // 16, 16).transpose(0, 2, 1),
                (1, 8, 1)))
    toff1_w = np.ascontiguousarray(
        toff1.reshape(NC_CORES, totch1, TILE_P).transpose(0, 2, 1)).astype(
            np.float16)
    tot2t = max(tot2, TILE_P)
    toff2_w = np.ascontiguousarray(
        toff2.reshape(NC_CORES, tot2t // TILE_P, TILE_P).transpose(0, 2, 1)
    ).astype(np.float16)

    dis_pad = np.zeros((NC_CORES, npc_pad), dtype=np.float32)
    dis_pad[:, :npc] = dis.reshape(NC_CORES, npc)
    dis_tiles = np.ascontiguousarray(
        dis_pad.reshape(NC_CORES, ntiles, TILE_P).transpose(0, 2, 1))
    dis2_tiles = np.ascontiguousarray(dis_tiles * dis_tiles)

    return dict(
        N=N, F=F, npc=npc, npc_pad=npc_pad, ntiles=ntiles, n_pad=n_pad,
        bounds_t=bounds_t, rows_p=rows_p, NP=NP,
        nch1=nch1, cs1=cs1, totch1=totch1,
        cP=cP, nch2=nch2, cs2=cs2, tot2=tot2,
        g1=g1.reshape(NC_CORES, 128, totch1 * F),
        xsl=xsl.reshape(NC_CORES, 128, ntiles * F),
        idx2=idx2_w, toff1=toff1_w, toff2=toff2_w,
        dis_tiles=dis_tiles, dis2_tiles=dis2_tiles,
    )


def _build_program(meta, has_b1, has_b2):
    import concourse.bacc as bacc
    import concourse.tile as tile
    from concourse import mybir

    F = meta["F"]
    ntiles = meta["ntiles"]
    npc_pad = meta["npc_pad"]
    bounds_t, rows_p, NP = meta["bounds_t"], meta["rows_p"], meta["NP"]
    nch1, cs1, totch1 = meta["nch1"], meta["cs1"], meta["totch1"]
    cP, nch2, cs2 = meta["cP"], meta["nch2"], meta["cs2"]
    totw2 = max(meta["tot2"], 16) // 16
    totch2 = max(meta["tot2"], TILE_P) // TILE_P
    nf = F // TILE_P
    f32, f16, i16 = mybir.dt.float32, mybir.dt.float16, mybir.dt.int16
    AF = mybir.ActivationFunctionType

    nc = bacc.Bacc("TRN2", target_bir_lowering=False, debug=False,
                   num_devices=NC_CORES, num_swdge_queues=N_QUEUES)

    g1_d = nc.dram_tensor("g1", [128, totch1 * F], f16, kind="ExternalInput")
    xsl_d = nc.dram_tensor("xsl", [128, ntiles * F], f16, kind="ExternalInput")
    idx_d = nc.dram_tensor("idx", [128, totw2], i16, kind="ExternalInput")
    toff1_d = nc.dram_tensor("toff1", [128, totch1], f16, kind="ExternalInput")
    toff2_d = nc.dram_tensor("toff2", [128, totch2], f16, kind="ExternalInput")
    dis_d = nc.dram_tensor("dis", [128, ntiles], f32, kind="ExternalInput")
    dis2_d = nc.dram_tensor("dis2", [128, ntiles], f32, kind="ExternalInput")
    w1_d = nc.dram_tensor("w1", [F, F], f16, kind="ExternalInput")
    w2_d = nc.dram_tensor("w2", [F, F], f16, kind="ExternalInput")
    id16_d = nc.dram_tensor("id16", [128, 128], f16, kind="ExternalInput")
    iota_d = nc.dram_tensor("iota", [128, 128], f16, kind="ExternalInput")
    if has_b1:
        b1_d = nc.dram_tensor("b1r", [128, F], f32, kind="ExternalInput")
    if has_b2:
        b2_d = nc.dram_tensor("b2r", [128, F], f32, kind="ExternalInput")
    out_d = nc.dram_tensor("out", [npc_pad, F], f16, kind="ExternalOutput")

    eq, add = mybir.AluOpType.is_equal, mybir.AluOpType.add

    with tile.TileContext(nc) as tc:
        with (
            tc.tile_pool(name="const", bufs=1) as cpool,
            tc.tile_pool(name="gbuf", bufs=4) as gpool,
            tc.tile_pool(name="pbuf", bufs=3) as ppool,
            tc.tile_pool(name="work", bufs=4) as wpool,
            tc.tile_pool(name="psA", bufs=3, space="PSUM") as psa,
            tc.tile_pool(name="psB", bufs=2, space="PSUM") as psb,
            tc.tile_pool(name="psC", bufs=3, space="PSUM") as psc,
            tc.tile_pool(name="dram", bufs=1, space="DRAM") as dpool,
        ):
            idx_sb = cpool.tile([128, totw2], i16)
            nc.sync.dma_start(idx_sb[:], idx_d[:, :])
            toff1_sb = cpool.tile([128, totch1], f16)
            nc.sync.dma_start(toff1_sb[:], toff1_d[:, :])
            toff2_sb = cpool.tile([128, totch2], f16)
            nc.sync.dma_start(toff2_sb[:], toff2_d[:, :])
            dis_sb = cpool.tile([128, ntiles], f32)
            nc.sync.dma_start(dis_sb[:], dis_d[:, :])
            dis2_sb = cpool.tile([128, ntiles], f32)
            nc.sync.dma_start(dis2_sb[:], dis2_d[:, :])
            id16_sb = cpool.tile([128, 128], f16)
            nc.sync.dma_start(id16_sb[:], id16_d[:, :])
            iota_sb = cpool.tile([128, 128], f16)
            nc.sync.dma_start(iota_sb[:], iota_d[:, :])
            w1_sb = cpool.tile([128, nf, F], f16)
            w2_sb = cpool.tile([128, nf, F], f16)
            for i in range(nf):
                nc.sync.dma_start(w1_sb[:, i, :], w1_d[128 * i:128 * (i + 1), :])
                nc.sync.dma_start(w2_sb[:, i, :], w2_d[128 * i:128 * (i + 1), :])
            if has_b1:
                b1_sb = cpool.tile([128, F], f32)
                nc.sync.dma_start(b1_sb[:], b1_d[:, :])
            if has_b2:
                b2_sb = cpool.tile([128, F], f32)
                nc.sync.dma_start(b2_sb[:], b2_d[:, :])

            # local shard, fp16: holds Dis*x during layer 1, then Dis*h1
            self_sb = cpool.tile([128, ntiles, F], f16)
            nc.sync.dma_start(
                self_sb[:], xsl_d[:, :].rearrange("p (t f) -> p t f", f=F))

            hs_shard = [dpool.tile([rows_p[p], F], f16, name=f"hs_shard{p}")
                        for p in range(NP)]
            hs_full = [dpool.tile([NC_CORES * rows_p[p], F], f16,
                                  addr_space="Shared", name=f"hs_full{p}")
                       for p in range(NP)]
            # piece -> tile index whose stage_b fires its AllGather
            ag_fire = {bounds_t[p + 1] - 1: p for p in range(NP)}

            def fire_ag(p):
                nc.gpsimd.collective_compute(
                    "AllGather", mybir.AluOpType.bypass,
                    replica_groups=[list(range(NC_CORES))],
                    ins=[hs_shard[p].opt()], outs=[hs_full[p].opt()])

            PRE = 3  # tiles whose piece<NP-1 gathers are issued pre-boundary
            g_pend = {}
            g_done = {}

            def gather_piece(t, p, G, qoff):
                cnt = int(cP[t, p])
                if cnt == 0:
                    return
                o_rel = int(cP[t, :p].sum())
                cs = int(cs2[t]) + o_rel
                nc.gpsimd.dma_gather(
                    G[:, o_rel:o_rel + cnt, :], hs_full[p][:, :],
                    idx_sb[:, cs * 8:(cs + cnt) * 8],
                    cnt * 128, cnt * 128, F,
                    single_packet=(cnt * 128 <= 1024),
                    queue_num=(NP * t + p + qoff) % N_QUEUES)

            for layer in range(2):
                w_sb = w1_sb if layer == 0 else w2_sb
                toff_sb = toff1_sb if layer == 0 else toff2_sb

                def build_p(t):
                    """one-hot matrices for tile t (VectorE), built one tile
                    ahead so the PE never waits on them."""
                    if layer == 0:
                        nch, cs = int(nch1[t]), int(cs1[t])
                    else:
                        nch, cs = int(nch2[t]), int(cs2[t])
                    if not nch:
                        return None
                    P = ppool.tile([128, nch, 128], f16, tag="P")
                    nc.vector.tensor_tensor(
                        P[:],
                        iota_sb[:].unsqueeze(1).broadcast_to([128, nch, 128]),
                        toff_sb[:, cs:cs + nch].unsqueeze(2).broadcast_to(
                            [128, nch, 128]),
                        eq)
                    return P

                def stage_a(t, P):
                    """gather/stream G, scatter-add the incoming messages +
                    self term into PSUM, copy to SBUF (ScalarE)."""
                    if layer == 0:
                        nch, cs = int(nch1[t]), int(cs1[t])
                        G = gpool.tile([128, max(nch, 1), F], f16, tag="G")
                        if nch:
                            nc.sync.dma_start(
                                G[:, 0:nch, :],
                                g1_d[:, cs * F:(cs + nch) * F].rearrange(
                                    "p (c f) -> p c f", f=F))
                    else:
                        nch = int(nch2[t])
                        if t in g_pend:
                            G = g_pend.pop(t)
                            done = g_done.pop(t)
                        else:
                            G = gpool.tile([128, max(nch, 1), F], f16,
                                           tag="G")
                            done = ()
                        for p in range(NP):
                            if p not in done:
                                gather_piece(t, p, G, 0)
                    # scatter-add (+ self term via identity weights)
                    aggp = psa.tile([128, F], f32, tag="aggp")
                    nc.tensor.matmul(aggp[:], id16_sb[:], self_sb[:, t, :],
                                     start=True, stop=(nch == 0))
                    for c in range(nch):
                        nc.tensor.matmul(aggp[:], P[:, c, :], G[:, c, :],
                                         start=False, stop=(c == nch - 1))
                    # PSUM -> SBUF f16 (ScalarE; Dis scaling folded into the
                    # final activation instead)
                    aggc = wpool.tile([128, F], f16, tag="aggc")
                    nc.scalar.activation(aggc[:], aggp[:], AF.Copy)
                    return aggc

                def stage_t(t, aggc):
                    """TensorE transpose of the aggregate + copy out of PSUM."""
                    pT = psb.tile([128, F], f16, tag="pT")
                    for i in range(nf):
                        nc.tensor.transpose(pT[:, 128 * i:128 * (i + 1)],
                                            aggc[:, 128 * i:128 * (i + 1)],
                                            id16_sb[:])
                    aggT = wpool.tile([128, nf, 128], f16, tag="aggT")
                    nc.vector.tensor_copy(
                        aggT[:].rearrange("p a b -> p (a b)"), pT[:])
                    return aggT

                def stage_b(t, aggT):
                    """dense weight matmul + scaled activation + writeback."""
                    zp = psc.tile([128, F], f32, tag="zp")
                    for i in range(nf):
                        nc.tensor.matmul(zp[:], aggT[:, i, :], w_sb[:, i, :],
                                         start=(i == 0), stop=(i == nf - 1))
                    r0, r1 = TILE_P * t, TILE_P * (t + 1)
                    if layer == 0:
                        zin = zp[:]
                        if has_b1:
                            zb = wpool.tile([128, F], f32, tag="zb")
                            nc.vector.tensor_tensor(zb[:], zp[:], b1_sb[:], add)
                            zin = zb[:]
                        # self_sb[t] := dis^2 * relu(z) == dis * relu(dis * z)
                        nc.scalar.activation(self_sb[:, t, :], zin, AF.Relu,
                                             scale=dis2_sb[:, t:t + 1])
                        # hs writes ride the (idle in layer 1) gpsimd queue so
                        # their waits never block the g1 stream on the SP ring
                        pc = int(np.searchsorted(
                            np.asarray(bounds_t[1:NP + 1]), t, side="right"))
                        b0 = r0 - bounds_t[pc] * TILE_P
                        nc.gpsimd.dma_start(hs_shard[pc][b0:b0 + TILE_P, :],
                                            self_sb[:, t, :])
                        if t in ag_fire:
                            k = ag_fire[t]
                            fire_ag(k)
                            # prefetch the first tiles' gathers from pieces
                            # whose AllGather has already completed.  G ring
                            # buffers may only be allocated once no further
                            # layer-1 allocations remain (else the 4-deep
                            # ring deadlocks against layer 1's own G tiles).
                            if k == NP - 2 and bounds_t[k + 1] >= ntiles - 1:
                                for tt in range(min(PRE, ntiles)):
                                    nch_t = int(nch2[tt])
                                    G = gpool.tile([128, max(nch_t, 1), F],
                                                   f16, tag="G")
                                    g_pend[tt] = G
                                    g_done[tt] = set()
                                    for p in range(NP - 2):
                                        gather_piece(tt, p, G, p)
                                        g_done[tt].add(p)
                            elif k == NP - 1 and NP >= 2:
                                if not g_pend:
                                    for tt in range(min(PRE, ntiles)):
                                        nch_t = int(nch2[tt])
                                        G = gpool.tile(
                                            [128, max(nch_t, 1), F],
                                            f16, tag="G")
                                        g_pend[tt] = G
                                        g_done[tt] = set()
                                for tt in sorted(g_pend):
                                    gather_piece(tt, NP - 2, g_pend[tt], 1)
                                    g_done[tt].add(NP - 2)
                    else:
                        o_t = wpool.tile([128, F], f16, tag="ot")
                        zin = zp[:]
                        if has_b2:
                            zb = wpool.tile([128, F], f32, tag="zb")
                            nc.vector.tensor_tensor(zb[:], zp[:], b2_sb[:], add)
                            zin = zb[:]
                        # out := dis * z  (SP ring is idle in layer 2)
                        nc.scalar.activation(o_t[:], zin, AF.Copy,
                                             scale=dis_sb[:, t:t + 1])
                        nc.sync.dma_start(out_d[r0:r1, :], o_t[:])

                # 2-deep software pipeline: PE order is aggp(t), T(t-1),
                # zp(t-2) so the TensorE stream never stalls on the
                # cross-engine transpose round-trip; P built one tile ahead
                p_next = build_p(0)
                aggc_q, aggt_q = {}, {}
                for t in range(ntiles + 2):
                    if t < ntiles:
                        P_cur = p_next
                        p_next = build_p(t + 1) if t + 1 < ntiles else None
                        aggc_q[t] = stage_a(t, P_cur)
                    if 1 <= t <= ntiles:
                        aggt_q[t - 1] = stage_t(t - 1, aggc_q.pop(t - 1))
                    if t >= 2:
                        stage_b(t - 2, aggt_q.pop(t - 2))

    nc.compile()
    return nc


def kernel(x, edge_index, W1, b1, W2, b2):
    x = np.asarray(x, dtype=np.float32)
    W1 = np.asarray(W1, dtype=np.float32)
    W2 = np.asarray(W2, dtype=np.float32)
    b1 = np.asarray(b1, dtype=np.float32)
    b2 = np.asarray(b2, dtype=np.float32)
    meta = _prep_host(x, edge_index)

    has_b1 = bool(np.any(b1))
    has_b2 = bool(np.any(b2))
    nc = _build_program(meta, has_b1, has_b2)

    in_maps = []
    for p in range(NC_CORES):
        m = {
            "g1": meta["g1"][p],
            "xsl": meta["xsl"][p],
            "idx": meta["idx2"][p],
            "toff1": meta["toff1"][p],
            "toff2": meta["toff2"][p],
            "dis": meta["dis_tiles"][p],
            "dis2": meta["dis2_tiles"][p],
            "w1": W1.astype(np.float16), "w2": W2.astype(np.float16),
            "id16": np.eye(128, dtype=np.float16),
            "iota": np.tile(np.arange(128, dtype=np.float16), (128, 1)),
        }
        if has_b1:
            m["b1r"] = np.tile(b1, (128, 1)).astype(np.float32)
        if has_b2:
            m["b2r"] = np.tile(b2, (128, 1)).astype(np.float32)
        in_maps.append(m)

    if os.environ.get("GNN_SIM", "0") == "1":
        from concourse.bass_interp import MultiCoreSim
        sim = MultiCoreSim(nc, num_cores=NC_CORES, trace=False)
        cores = list(sim.cores.values())
        for p, core in enumerate(cores):
            for k, v in in_maps[p].items():
                core.tensor(k)[:] = v
        sim.simulate(check_with_hw=False)
        shards = [cores[p].tensor("out").copy() for p in range(NC_CORES)]
    else:
        from concourse import bass_utils
        trace = os.environ.get("GNN_TRACE", "0") == "1"
        res = bass_utils.run_bass_kernel_spmd(
            nc, in_maps, core_ids=list(range(NC_CORES)), trace=trace)
        if trace and res.exec_time_ns is not None:
            print(f"HW exec time: {res.exec_time_ns} ns")
        kernel.last_results = res
        shards = [res.results[p]["out"] for p in range(NC_CORES)]

    npc = meta["npc"]
    out = np.concatenate([s[:npc] for s in shards], axis=0)
    return out.astype(np.float32)



# revision 5
# speedup vs baseline: 1.4605x; 1.4605x over previous
"""Trainium2 Bass kernel: 2-layer GCN (GCNConv -> ReLU -> GCNConv).

Math:  S = D^-1/2 (A + I) D^-1/2  (A from edge_index, self-loops appended)
       out = S @ relu(S @ x @ W1 + b1) @ W2 + b2
Using linearity, aggregate-then-matmul per layer with u = Dis*x:
       agg1 = A' @ u + u            (A' = adjacency without self-loops)
       h~   = Dis^2 * relu(agg1 @ W1 (+ b1))   (= Dis * h1, stored fp16)
       agg2 = A' @ h~ + h~
       out  = Dis * (agg2 @ W2) (+ b2)
The Dis row-scaling commutes with the dense matmul, so it is folded into a
single ScalarE activation per tile (scale = dis^2 resp. dis, func=relu/copy).

Distribution: nodes sharded over 8 NeuronCores.  Per layer, each core
scatter-adds incoming-edge source rows per 128-target tile with TensorE
matmuls against one-hot matrices (built on VectorE via is_equal vs iota, one
tile ahead of use), transposes via TensorE and applies the dense weight
matmul in fp16, all on a 2-deep software pipeline (PE order: aggregate(t),
transpose(t-1), dense(t-2)) so the TensorE stream never stalls.

Precision: the gathered/streamed neighbor rows, the one-hot matrices and the
AllGather payload are fp8e4 (self term, weights and dense path stay fp16;
measured rel-err 8e-3 vs the 2e-2 gate).  The scatter-add runs two fp8
chunks per PE pass with MatmulPerfMode.DoubleRow (2x fp8 throughput), so
both HBM traffic and aggregation PE time halve vs fp16.
  Layer 1: gather indices are static and the source data (fp8 Dis*x) is a
  kernel input, so the HOST pre-expands the gathered stream into edge order;
  the device streams it sequentially over HWDGE at full HBM bandwidth.
  Layer 2: fp8 activations are AllGathered in FIVE tapered pieces
  ([0,20,34,44,48,49] tile bounds) fired as soon as each piece's tiles are
  computed, so the collective pipelines behind layer 1 and only the tiny
  1-tile final piece is exposed at the layer boundary; rows are then fetched
  per-edge with gpsimd dma_gather (512B descriptors).  Descriptor generation
  is spread over all 4 SWDGE queues; gather indices are pre-sorted for HBM
  locality.
"""

import os
import numpy as np
import ml_dtypes

NC_CORES = 8
TILE_P = 128
N_QUEUES = 4
FP8 = ml_dtypes.float8_e4m3


def _round_up(v, m):
    return (v + m - 1) // m * m


def _piece_bounds(ntiles):
    """Tapered AllGather piece boundaries (in tiles): large early pieces
    hide behind layer-1 compute, tiny final piece minimizes the exposed
    latency at the layer boundary.  Each piece must stay int16-indexable:
    8 * piece_tiles * 128 <= 32767  ->  piece_tiles <= 31."""
    cuts = (0.41, 0.70, 0.90, 0.98)
    b = sorted({0, ntiles, *(min(int(ntiles * c), ntiles) for c in cuts)})
    out = [b[0]]
    for x in b[1:]:
        while x - out[-1] > 31:
            out.append(out[-1] + 31)
        out.append(x)
    return out


def _prep_host(x, edge_index):
    """Partition + pad the graph; build per-core stream/gather metadata."""
    x = np.asarray(x, dtype=np.float32)
    edge_index = np.asarray(edge_index)
    N, F = x.shape
    assert N % NC_CORES == 0, (N, NC_CORES)
    npc = N // NC_CORES
    npc_pad = _round_up(npc, TILE_P)
    ntiles = npc_pad // TILE_P
    n_pad = NC_CORES * npc_pad

    loops = np.arange(N, dtype=np.int64)
    # edges WITHOUT self-loops (self term handled on-device)
    row = edge_index[0].astype(np.int64)
    col = edge_index[1].astype(np.int64)
    # degree WITH self-loops (as the reference computes it)
    deg = np.bincount(np.concatenate([col, loops]), minlength=N).astype(np.float64)
    dis = np.where(deg > 0, 1.0 / np.sqrt(deg), 0.0).astype(np.float32)

    src_pad = (row // npc) * npc_pad + (row % npc)
    tgt_core = (col // npc).astype(np.int64)
    tgt_loc = col % npc
    tile_of = tgt_loc // TILE_P
    toff_of = (tgt_loc % TILE_P).astype(np.float32)

    # ---- layer 1: host-expanded stream, grouped by (core, tile) ----
    key1 = tgt_core * ntiles + tile_of
    cnt1 = np.bincount(key1, minlength=NC_CORES * ntiles).reshape(
        NC_CORES, ntiles)
    C1 = _round_up(cnt1.max(axis=0), TILE_P)  # padded counts [ntiles]
    tot1 = int(C1.sum())
    totch1 = tot1 // TILE_P
    nch1 = (C1 // TILE_P).astype(np.int64)
    cs1 = np.zeros(ntiles, dtype=np.int64)
    np.cumsum(nch1[:-1], out=cs1[1:])

    o1 = np.argsort(key1, kind="stable")
    src1_s, toff1_s = src_pad[o1], toff_of[o1]
    g1start = np.zeros(NC_CORES * ntiles + 1, dtype=np.int64)
    np.cumsum(cnt1.reshape(-1), out=g1start[1:])

    # ---- layer 2: dma_gather, grouped by (core, tile, piece) ----
    bounds_t = _piece_bounds(ntiles)
    NP = len(bounds_t) - 1
    bounds_r = [b * TILE_P for b in bounds_t]
    rows_p = [bounds_r[i + 1] - bounds_r[i] for i in range(NP)]
    for r in rows_p:
        assert NC_CORES * r <= 32767, (rows_p,)

    src_core = row // npc
    src_loc = row % npc
    piece = np.searchsorted(np.asarray(bounds_r[1:NP]), src_loc, side="right")
    key2 = key1 * NP + piece
    cnt2 = np.bincount(key2, minlength=NC_CORES * ntiles * NP).reshape(
        NC_CORES, ntiles, NP)
    C2 = cnt2.max(axis=0)
    C2 = np.where(C2 > 0, _round_up(C2, TILE_P), 0)  # [ntiles, NP]
    tot2 = int(C2.sum())
    cP = (C2 // TILE_P).astype(np.int64)  # chunks per (tile, piece)
    nch2 = cP.sum(axis=1)
    cs2 = np.zeros(ntiles, dtype=np.int64)
    np.cumsum(nch2[:-1], out=cs2[1:])

    start_r = np.asarray([bounds_r[p] for p in range(NP)])
    rows_arr = np.asarray(rows_p)
    piece_idx = src_core * rows_arr[piece] + (src_loc - start_r[piece])
    # secondary sort by source index: the one-hot P absorbs any within-group
    # permutation, and ascending gather addresses improve HBM locality
    o2 = np.lexsort((piece_idx, key2))
    src2_s, toff2_s = piece_idx[o2], toff_of[o2]
    g2start = np.zeros(NC_CORES * ntiles * NP + 1, dtype=np.int64)
    np.cumsum(cnt2.reshape(-1), out=g2start[1:])

    # padded Dis*x: fp8 copy feeds the edge streams, fp16 the self term
    xs32 = dis[:, None] * x
    xs_pad8 = np.zeros((NC_CORES, npc_pad, F), dtype=FP8)
    xs_pad8[:, :npc] = xs32.reshape(NC_CORES, npc, F).astype(FP8)
    xs_pad16 = np.zeros((NC_CORES, npc_pad, F), dtype=np.float16)
    xs_pad16[:, :npc] = xs32.reshape(NC_CORES, npc, F).astype(np.float16)
    xsl = np.ascontiguousarray(
        xs_pad16.reshape(NC_CORES, ntiles, TILE_P, F).transpose(0, 2, 1, 3))
    xs_flat8 = xs_pad8.reshape(n_pad, F)

    g1 = np.zeros((NC_CORES, 128, totch1, F), dtype=FP8)
    toff1 = np.full((NC_CORES, tot1), -1.0, dtype=np.float32)
    idx2 = np.zeros((NC_CORES, max(tot2, 16)), dtype=np.int16)
    toff2 = np.full((NC_CORES, max(tot2, TILE_P)), -1.0, dtype=np.float32)
    for p in range(NC_CORES):
        off = 0
        for t in range(ntiles):
            g = p * ntiles + t
            a, b = g1start[g], g1start[g + 1]
            n = b - a
            blk = g1[p, :, cs1[t]:cs1[t] + nch1[t], :]
            j = np.arange(n)
            # stream row j -> partition j%128, chunk j//128
            blk[j % 128, j // 128] = xs_flat8[src1_s[a:b]]
            toff1[p, off:off + n] = toff1_s[a:b]
            off += C1[t]
        off = 0
        for t in range(ntiles):
            for h in range(NP):
                c = int(C2[t, h])
                if c == 0:
                    continue
                g = (p * ntiles + t) * NP + h
                a, b = g2start[g], g2start[g + 1]
                n = b - a
                s = src2_s[a:b]
                assert n <= c and (s >= 0).all() and (s < 32767).all()
                idx2[p, off:off + n] = s.astype(np.int16)
                toff2[p, off:off + n] = toff2_s[a:b]
                off += c

    tot2c = max(tot2, 16)
    idx2_w = np.ascontiguousarray(
        np.tile(idx2.reshape(NC_CORES, tot2c // 16, 16).transpose(0, 2, 1),
                (1, 8, 1)))
    toff1_w = np.ascontiguousarray(
        toff1.reshape(NC_CORES, totch1, TILE_P).transpose(0, 2, 1)).astype(
            np.float16)
    tot2t = max(tot2, TILE_P)
    toff2_w = np.ascontiguousarray(
        toff2.reshape(NC_CORES, tot2t // TILE_P, TILE_P).transpose(0, 2, 1)
    ).astype(np.float16)

    dis_pad = np.zeros((NC_CORES, npc_pad), dtype=np.float32)
    dis_pad[:, :npc] = dis.reshape(NC_CORES, npc)
    dis_tiles = np.ascontiguousarray(
        dis_pad.reshape(NC_CORES, ntiles, TILE_P).transpose(0, 2, 1))
    dis2_tiles = np.ascontiguousarray(dis_tiles * dis_tiles)

    return dict(
        N=N, F=F, npc=npc, npc_pad=npc_pad, ntiles=ntiles, n_pad=n_pad,
        bounds_t=bounds_t, rows_p=rows_p, NP=NP,
        nch1=nch1, cs1=cs1, totch1=totch1,
        cP=cP, nch2=nch2, cs2=cs2, tot2=tot2,
        g1=g1.reshape(NC_CORES, 128, totch1 * F),
        xsl=xsl.reshape(NC_CORES, 128, ntiles * F),
        idx2=idx2_w, toff1=toff1_w, toff2=toff2_w,
        dis_tiles=dis_tiles, dis2_tiles=dis2_tiles,
    )


def _build_program(meta, has_b1, has_b2):
    import concourse.bacc as bacc
    import concourse.tile as tile
    from concourse import mybir

    F = meta["F"]
    ntiles = meta["ntiles"]
    npc_pad = meta["npc_pad"]
    bounds_t, rows_p, NP = meta["bounds_t"], meta["rows_p"], meta["NP"]
    nch1, cs1, totch1 = meta["nch1"], meta["cs1"], meta["totch1"]
    cP, nch2, cs2 = meta["cP"], meta["nch2"], meta["cs2"]
    totw2 = max(meta["tot2"], 16) // 16
    totch2 = max(meta["tot2"], TILE_P) // TILE_P
    nf = F // TILE_P
    f32, f16, i16 = mybir.dt.float32, mybir.dt.float16, mybir.dt.int16
    f8 = mybir.dt.float8e4
    AF = mybir.ActivationFunctionType
    DR = mybir.MatmulPerfMode.DoubleRow

    nc = bacc.Bacc("TRN2", target_bir_lowering=False, debug=False,
                   num_devices=NC_CORES, num_swdge_queues=N_QUEUES)

    g1_d = nc.dram_tensor("g1", [128, totch1 * F], f8, kind="ExternalInput")
    xsl_d = nc.dram_tensor("xsl", [128, ntiles * F], f16, kind="ExternalInput")
    idx_d = nc.dram_tensor("idx", [128, totw2], i16, kind="ExternalInput")
    toff1_d = nc.dram_tensor("toff1", [128, totch1], f16, kind="ExternalInput")
    toff2_d = nc.dram_tensor("toff2", [128, totch2], f16, kind="ExternalInput")
    dis_d = nc.dram_tensor("dis", [128, ntiles], f32, kind="ExternalInput")
    dis2_d = nc.dram_tensor("dis2", [128, ntiles], f32, kind="ExternalInput")
    w1_d = nc.dram_tensor("w1", [F, F], f16, kind="ExternalInput")
    w2_d = nc.dram_tensor("w2", [F, F], f16, kind="ExternalInput")
    id16_d = nc.dram_tensor("id16", [128, 128], f16, kind="ExternalInput")
    iota_d = nc.dram_tensor("iota", [128, 128], f16, kind="ExternalInput")
    if has_b1:
        b1_d = nc.dram_tensor("b1r", [128, F], f32, kind="ExternalInput")
    if has_b2:
        b2_d = nc.dram_tensor("b2r", [128, F], f32, kind="ExternalInput")
    out_d = nc.dram_tensor("out", [npc_pad, F], f16, kind="ExternalOutput")

    eq, add = mybir.AluOpType.is_equal, mybir.AluOpType.add

    with tile.TileContext(nc) as tc:
        with (
            tc.tile_pool(name="const", bufs=1) as cpool,
            tc.tile_pool(name="gbuf", bufs=4) as gpool,
            tc.tile_pool(name="pbuf", bufs=3) as ppool,
            tc.tile_pool(name="work", bufs=4) as wpool,
            tc.tile_pool(name="h8buf", bufs=3) as hpool,
            tc.tile_pool(name="psA", bufs=3, space="PSUM") as psa,
            tc.tile_pool(name="psB", bufs=2, space="PSUM") as psb,
            tc.tile_pool(name="psC", bufs=3, space="PSUM") as psc,
            tc.tile_pool(name="dram", bufs=1, space="DRAM") as dpool,
        ):
            idx_sb = cpool.tile([128, totw2], i16)
            nc.sync.dma_start(idx_sb[:], idx_d[:, :])
            toff1_sb = cpool.tile([128, totch1], f16)
            nc.sync.dma_start(toff1_sb[:], toff1_d[:, :])
            toff2_sb = cpool.tile([128, totch2], f16)
            nc.sync.dma_start(toff2_sb[:], toff2_d[:, :])
            dis_sb = cpool.tile([128, ntiles], f32)
            nc.sync.dma_start(dis_sb[:], dis_d[:, :])
            dis2_sb = cpool.tile([128, ntiles], f32)
            nc.sync.dma_start(dis2_sb[:], dis2_d[:, :])
            id16_sb = cpool.tile([128, 128], f16)
            nc.sync.dma_start(id16_sb[:], id16_d[:, :])
            iota_sb = cpool.tile([128, 128], f16)
            nc.sync.dma_start(iota_sb[:], iota_d[:, :])
            w1_sb = cpool.tile([128, nf, F], f16)
            w2_sb = cpool.tile([128, nf, F], f16)
            for i in range(nf):
                nc.sync.dma_start(w1_sb[:, i, :], w1_d[128 * i:128 * (i + 1), :])
                nc.sync.dma_start(w2_sb[:, i, :], w2_d[128 * i:128 * (i + 1), :])
            if has_b1:
                b1_sb = cpool.tile([128, F], f32)
                nc.sync.dma_start(b1_sb[:], b1_d[:, :])
            if has_b2:
                b2_sb = cpool.tile([128, F], f32)
                nc.sync.dma_start(b2_sb[:], b2_d[:, :])

            # local shard, fp16: holds Dis*x during layer 1, then Dis*h1
            self_sb = cpool.tile([128, ntiles, F], f16)
            nc.sync.dma_start(
                self_sb[:], xsl_d[:, :].rearrange("p (t f) -> p t f", f=F))

            hs_shard = [dpool.tile([rows_p[p], F], f8, name=f"hs_shard{p}")
                        for p in range(NP)]
            hs_full = [dpool.tile([NC_CORES * rows_p[p], F], f8,
                                  addr_space="Shared", name=f"hs_full{p}")
                       for p in range(NP)]
            # piece -> tile index whose stage_b fires its AllGather
            ag_fire = {bounds_t[p + 1] - 1: p for p in range(NP)}

            def fire_ag(p):
                nc.gpsimd.collective_compute(
                    "AllGather", mybir.AluOpType.bypass,
                    replica_groups=[list(range(NC_CORES))],
                    ins=[hs_shard[p].opt()], outs=[hs_full[p].opt()])

            PRE = 3  # tiles whose gathers are issued before the main loop
            g_pend = {}
            g_done = {}
            # SWDGE queue must advance in lockstep with issue order: the tile
            # scheduler hands out DMASW sem lanes round-robin per SWDGE
            # instruction, and each sem is locked to one queue — a strict
            # global cycle keeps lane<->queue consistent.
            gq = [0]

            def gather_piece(t, p, G):
                cnt = int(cP[t, p])
                if cnt == 0:
                    return
                o_rel = int(cP[t, :p].sum())
                cs = int(cs2[t]) + o_rel
                q = gq[0] % N_QUEUES
                gq[0] += 1
                nc.gpsimd.dma_gather(
                    G[:, o_rel:o_rel + cnt, :], hs_full[p][:, :],
                    idx_sb[:, cs * 8:(cs + cnt) * 8],
                    cnt * 128, cnt * 128, F,
                    single_packet=(cnt * 128 <= 128),
                    queue_num=q)

            def agg_matmuls(aggp, P, G, t, nch):
                """scatter-add: self term (fp16) + fp8 DoubleRow chunk pairs."""
                nc.tensor.matmul(aggp[:], id16_sb[:], self_sb[:, t, :],
                                 start=True, stop=(nch == 0))
                c = 0
                while c < nch:
                    if c + 2 <= nch:
                        nc.tensor.matmul(aggp[:], P[:, c:c + 2, :],
                                         G[:, c:c + 2, :], start=False,
                                         stop=(c + 2 == nch), perf_mode=DR)
                        c += 2
                    else:
                        nc.tensor.matmul(aggp[:], P[:, c, :], G[:, c, :],
                                         start=False, stop=True)
                        c += 1

            for layer in range(2):
                w_sb = w1_sb if layer == 0 else w2_sb
                toff_sb = toff1_sb if layer == 0 else toff2_sb

                def build_p(t):
                    """one-hot matrices for tile t (VectorE), built one tile
                    ahead so the PE never waits on them."""
                    if layer == 0:
                        nch, cs = int(nch1[t]), int(cs1[t])
                    else:
                        nch, cs = int(nch2[t]), int(cs2[t])
                    if not nch:
                        return None
                    P = ppool.tile([128, nch, 128], f8, tag="P")
                    nc.vector.tensor_tensor(
                        P[:],
                        iota_sb[:].unsqueeze(1).broadcast_to([128, nch, 128]),
                        toff_sb[:, cs:cs + nch].unsqueeze(2).broadcast_to(
                            [128, nch, 128]),
                        eq)
                    return P

                def stage_a(t, P):
                    """gather/stream G, scatter-add the incoming messages +
                    self term into PSUM, copy to SBUF (ScalarE)."""
                    if layer == 0:
                        nch, cs = int(nch1[t]), int(cs1[t])
                        G = gpool.tile([128, max(nch, 1), F], f8, tag="G")
                        if nch:
                            nc.sync.dma_start(
                                G[:, 0:nch, :],
                                g1_d[:, cs * F:(cs + nch) * F].rearrange(
                                    "p (c f) -> p c f", f=F))
                    else:
                        nch = int(nch2[t])
                        if t in g_pend:
                            G = g_pend.pop(t)
                            done = g_done.pop(t)
                        else:
                            G = gpool.tile([128, max(nch, 1), F], f8,
                                           tag="G")
                            done = ()
                        for p in range(NP):
                            if p not in done:
                                gather_piece(t, p, G)
                    # scatter-add (+ self term via identity weights)
                    aggp = psa.tile([128, F], f32, tag="aggp")
                    agg_matmuls(aggp, P, G, t, nch)
                    # PSUM -> SBUF f16 (ScalarE; Dis scaling folded into the
                    # final activation instead)
                    aggc = wpool.tile([128, F], f16, tag="aggc")
                    nc.scalar.activation(aggc[:], aggp[:], AF.Copy)
                    return aggc

                def stage_t(t, aggc):
                    """TensorE transpose of the aggregate + copy out of PSUM."""
                    pT = psb.tile([128, F], f16, tag="pT")
                    for i in range(nf):
                        nc.tensor.transpose(pT[:, 128 * i:128 * (i + 1)],
                                            aggc[:, 128 * i:128 * (i + 1)],
                                            id16_sb[:])
                    aggT = wpool.tile([128, nf, 128], f16, tag="aggT")
                    nc.vector.tensor_copy(
                        aggT[:].rearrange("p a b -> p (a b)"), pT[:])
                    return aggT

                def stage_b(t, aggT):
                    """dense weight matmul + scaled activation + writeback."""
                    zp = psc.tile([128, F], f32, tag="zp")
                    for i in range(nf):
                        nc.tensor.matmul(zp[:], aggT[:, i, :], w_sb[:, i, :],
                                         start=(i == 0), stop=(i == nf - 1))
                    r0, r1 = TILE_P * t, TILE_P * (t + 1)
                    if layer == 0:
                        zin = zp[:]
                        if has_b1:
                            zb = wpool.tile([128, F], f32, tag="zb")
                            nc.vector.tensor_tensor(zb[:], zp[:], b1_sb[:], add)
                            zin = zb[:]
                        # self_sb[t] := dis^2 * relu(z) == dis * relu(dis * z)
                        nc.scalar.activation(self_sb[:, t, :], zin, AF.Relu,
                                             scale=dis2_sb[:, t:t + 1])
                        # fp8 copy of the same activation for the AllGather /
                        # layer-2 gather stream
                        h8 = hpool.tile([128, F], f8, tag="h8")
                        nc.scalar.activation(h8[:], zin, AF.Relu,
                                             scale=dis2_sb[:, t:t + 1])
                        pc = int(np.searchsorted(
                            np.asarray(bounds_t[1:NP + 1]), t, side="right"))
                        b0 = r0 - bounds_t[pc] * TILE_P
                        # hs writes ride the ScalarE HWDGE queue: off the
                        # SWDGE lanes (whose sem rotation the gathers own)
                        # and off the SP ring (so the g1 stream never waits)
                        nc.scalar.dma_start(hs_shard[pc][b0:b0 + TILE_P, :],
                                            h8[:])
                        if t in ag_fire:
                            k = ag_fire[t]
                            fire_ag(k)
                            # prefetch the first tiles' gathers from pieces
                            # whose AllGather has already completed.  G ring
                            # buffers may only be allocated once no further
                            # layer-1 allocations remain (else the 4-deep
                            # ring deadlocks against layer 1's own G tiles).
                            # Pieces close to the just-fired AG are deferred
                            # to the final firing so their semaphore waits
                            # never stall the gpsimd engine before the last
                            # AllGathers are triggered.
                            if k == NP - 2 and NP >= 2 and \
                                    bounds_t[k + 1] >= ntiles - 1:
                                for tt in range(min(PRE, ntiles)):
                                    nch_t = int(nch2[tt])
                                    G = gpool.tile([128, max(nch_t, 1), F],
                                                   f8, tag="G")
                                    g_pend[tt] = G
                                    g_done[tt] = set()
                                    for p in range(max(NP - 3, 0)):
                                        gather_piece(tt, p, G)
                                        g_done[tt].add(p)
                            elif k == NP - 1:
                                if not g_pend:
                                    for tt in range(min(PRE, ntiles)):
                                        nch_t = int(nch2[tt])
                                        G = gpool.tile(
                                            [128, max(nch_t, 1), F],
                                            f8, tag="G")
                                        g_pend[tt] = G
                                        g_done[tt] = set()
                                for tt in sorted(g_pend):
                                    for p in range(max(NP - 3, 0), NP):
                                        if p not in g_done[tt]:
                                            gather_piece(tt, p, g_pend[tt])
                                            g_done[tt].add(p)
                    else:
                        o_t = wpool.tile([128, F], f16, tag="ot")
                        zin = zp[:]
                        if has_b2:
                            zb = wpool.tile([128, F], f32, tag="zb")
                            nc.vector.tensor_tensor(zb[:], zp[:], b2_sb[:], add)
                            zin = zb[:]
                        # out := dis * z  (SP ring is idle in layer 2)
                        nc.scalar.activation(o_t[:], zin, AF.Copy,
                                             scale=dis_sb[:, t:t + 1])
                        nc.sync.dma_start(out_d[r0:r1, :], o_t[:])

                # 2-deep software pipeline: PE order is aggp(t), T(t-1),
                # zp(t-2) so the TensorE stream never stalls on the
                # cross-engine transpose round-trip; P built one tile ahead
                p_next = build_p(0)
                aggc_q, aggt_q = {}, {}
                for t in range(ntiles + 2):
                    if t < ntiles:
                        P_cur = p_next
                        p_next = build_p(t + 1) if t + 1 < ntiles else None
                        aggc_q[t] = stage_a(t, P_cur)
                    if 1 <= t <= ntiles:
                        aggt_q[t - 1] = stage_t(t - 1, aggc_q.pop(t - 1))
                    if t >= 2:
                        stage_b(t - 2, aggt_q.pop(t - 2))

    nc.compile()
    return nc


def kernel(x, edge_index, W1, b1, W2, b2):
    x = np.asarray(x, dtype=np.float32)
    W1 = np.asarray(W1, dtype=np.float32)
    W2 = np.asarray(W2, dtype=np.float32)
    b1 = np.asarray(b1, dtype=np.float32)
    b2 = np.asarray(b2, dtype=np.float32)
    meta = _prep_host(x, edge_index)

    has_b1 = bool(np.any(b1))
    has_b2 = bool(np.any(b2))
    nc = _build_program(meta, has_b1, has_b2)

    in_maps = []
    for p in range(NC_CORES):
        m = {
            "g1": meta["g1"][p],
            "xsl": meta["xsl"][p],
            "idx": meta["idx2"][p],
            "toff1": meta["toff1"][p],
            "toff2": meta["toff2"][p],
            "dis": meta["dis_tiles"][p],
            "dis2": meta["dis2_tiles"][p],
            "w1": W1.astype(np.float16), "w2": W2.astype(np.float16),
            "id16": np.eye(128, dtype=np.float16),
            "iota": np.tile(np.arange(128, dtype=np.float16), (128, 1)),
        }
        if has_b1:
            m["b1r"] = np.tile(b1, (128, 1)).astype(np.float32)
        if has_b2:
            m["b2r"] = np.tile(b2, (128, 1)).astype(np.float32)
        in_maps.append(m)

    if os.environ.get("GNN_SIM", "0") == "1":
        from concourse.bass_interp import MultiCoreSim
        sim = MultiCoreSim(nc, num_cores=NC_CORES, trace=False)
        cores = list(sim.cores.values())
        for p, core in enumerate(cores):
            for k, v in in_maps[p].items():
                core.tensor(k)[:] = v
        sim.simulate(check_with_hw=False)
        shards = [cores[p].tensor("out").copy() for p in range(NC_CORES)]
    else:
        from concourse import bass_utils
        trace = os.environ.get("GNN_TRACE", "0") == "1"
        res = bass_utils.run_bass_kernel_spmd(
            nc, in_maps, core_ids=list(range(NC_CORES)), trace=trace)
        if trace and res.exec_time_ns is not None:
            print(f"HW exec time: {res.exec_time_ns} ns")
        kernel.last_results = res
        shards = [res.results[p]["out"] for p in range(NC_CORES)]

    npc = meta["npc"]
    out = np.concatenate([s[:npc] for s in shards], axis=0)
    return out.astype(np.float32)
